# revision 1
# baseline (speedup 1.0000x reference)
"""Trainium2 Bass kernel for CrossAttentionFusion.

Reference computation (per sample b):
    q = Wq @ yolo + bq            [32, N]    (N = 64*64 = 4096)
    k = Wk @ vit + bk             [32, N]
    v = Wv @ vit + bv             [256, N]
    A = softmax((q^T k) / sqrt(32), axis=j)         [N, N]
    out = yolo + Wo @ (v @ A^T) + bo                [256, N]

Sharding: data-parallel over batch B=8 across 8 cores; weights replicated.

Device algorithm (per core, one sample), all matmuls bf16 with fp32 PSUM
accumulation:
  - attnT[j, i] = k^T q computed directly in [j, i] orientation so that both
    the softmax denominator and the A·V contraction reduce over the PSUM
    partition axis (no N x N transpose needed).
  - P = exp(scale * attnT) without max subtraction (|logits| < 1 for this
    problem's scale statistics; exp is safe).
  - U[c, i] = sum_j vT[j, c] P[j, i]  (unnormalized), denom[i] = sum_j P[j, i]
    via an all-ones stationary matmul accumulated alongside.
  - softmax normalization commutes through the output projection:
    out = yolo + (Wo @ U) * (1/denom) + (Wo @ bv + bo).
  - v^T is produced directly by the V-projection (lhsT = vit chunk), so no
    transposes anywhere.
"""

import sys

sys.path.insert(0, "/opt/trn_rl_repo")

import numpy as np
import ml_dtypes

import concourse.bass as bass
import concourse.tile as tile
from concourse import bacc, mybir
from concourse.bass_utils import run_bass_kernel_spmd

BF16 = ml_dtypes.bfloat16
F32 = mybir.dt.float32
BF = mybir.dt.bfloat16

B, C, H, W = 8, 256, 64, 64
N = H * W            # 4096
CQK = C // 8         # 32
P = 128              # partitions
IB = 512             # i-block (one PSUM bank of fp32)
NIB = N // IB        # 8
JT = N // P          # 32 j-tiles
CC = C // P          # 2 channel chunks
SCALE = 1.0 / float(np.sqrt(np.float32(CQK)))


def build_nc():
    nc = bacc.Bacc("TRN2", target_bir_lowering=False, debug=False)

    x_yolo = nc.dram_tensor("x_yolo", [C, N], F32, kind="ExternalInput")
    x_vit = nc.dram_tensor("x_vit", [C, N], F32, kind="ExternalInput")
    wqt = nc.dram_tensor("wqt", [C, CQK], BF, kind="ExternalInput")
    wkt = nc.dram_tensor("wkt", [C, CQK], BF, kind="ExternalInput")
    wvt = nc.dram_tensor("wvt", [C, C], BF, kind="ExternalInput")
    wot = nc.dram_tensor("wot", [C, C], BF, kind="ExternalInput")
    bq2 = nc.dram_tensor("bq2", [CQK, 1], F32, kind="ExternalInput")
    bk2 = nc.dram_tensor("bk2", [CQK, 1], F32, kind="ExternalInput")
    bop = nc.dram_tensor("bop", [C, 1], F32, kind="ExternalInput")
    out = nc.dram_tensor("out", [C, N], F32, kind="ExternalOutput")

    with tile.TileContext(nc) as tc:
        with (
            tc.tile_pool(name="singles", bufs=1) as sg,
            tc.tile_pool(name="pp", bufs=3) as pp,
            tc.tile_pool(name="pu", bufs=4) as pu,
            tc.tile_pool(name="pr", bufs=2) as pr,
            tc.tile_pool(name="po", bufs=4) as po,
            tc.tile_pool(name="ps_mm", bufs=3, space="PSUM") as ps_mm,
            tc.tile_pool(name="ps_u", bufs=2, space="PSUM") as ps_u,
            tc.tile_pool(name="ps_den", bufs=2, space="PSUM") as ps_den,
        ):
            # ---- Phase A: loads, casts, residual-with-bias precompute ----
            xy = []
            xv = []
            xy_bf = []
            xv_bf = []
            for cc in range(CC):
                t = sg.tile([P, N], F32, name=f"xy{cc}")
                nc.sync.dma_start(t[:], x_yolo[cc * P : (cc + 1) * P, :])
                xy.append(t)
                t = sg.tile([P, N], F32, name=f"xv{cc}")
                nc.sync.dma_start(t[:], x_vit[cc * P : (cc + 1) * P, :])
                xv.append(t)

            wqt_sb = []
            wkt_sb = []
            wvt_sb = []
            wot_sb = []
            for cc in range(CC):
                t = sg.tile([P, CQK], BF, name=f"wqt{cc}")
                nc.sync.dma_start(t[:], wqt[cc * P : (cc + 1) * P, :])
                wqt_sb.append(t)
                t = sg.tile([P, CQK], BF, name=f"wkt{cc}")
                nc.sync.dma_start(t[:], wkt[cc * P : (cc + 1) * P, :])
                wkt_sb.append(t)
                t = sg.tile([P, C], BF, name=f"wvt{cc}")
                nc.sync.dma_start(t[:], wvt[cc * P : (cc + 1) * P, :])
                wvt_sb.append(t)
                t = sg.tile([P, C], BF, name=f"wot{cc}")
                nc.sync.dma_start(t[:], wot[cc * P : (cc + 1) * P, :])
                wot_sb.append(t)

            bq_sb = sg.tile([CQK, 1], F32)
            nc.sync.dma_start(bq_sb[:], bq2[:])
            bk_sb = sg.tile([CQK, 1], F32)
            nc.sync.dma_start(bk_sb[:], bk2[:])
            bop_sb = []
            for cc in range(CC):
                t = sg.tile([P, 1], F32, name=f"bop{cc}")
                nc.sync.dma_start(t[:], bop[cc * P : (cc + 1) * P, :])
                bop_sb.append(t)

            ones_sb = sg.tile([P, P], BF)
            nc.vector.memset(ones_sb[:], 1.0)

            for cc in range(CC):
                t = sg.tile([P, N], BF, name=f"xybf{cc}")
                nc.vector.tensor_copy(t[:], xy[cc][:])
                xy_bf.append(t)
                t = sg.tile([P, N], BF, name=f"xvbf{cc}")
                nc.vector.tensor_copy(t[:], xv[cc][:])
                xv_bf.append(t)
                # xy becomes yb = yolo + (Wo @ bv + bo), the epilogue addend
                nc.vector.tensor_scalar_add(
                    out=xy[cc][:], in0=xy[cc][:], scalar1=bop_sb[cc][:]
                )

            # ---- Phase B: q/k projections ----
            q_sb = sg.tile([CQK, N], BF)
            k_sb = sg.tile([CQK, N], BF)
            for ic in range(NIB):
                isl = slice(ic * IB, (ic + 1) * IB)
                q_ps = ps_mm.tile([CQK, IB], F32, tag="mm", name="q_ps")
                for cc in range(CC):
                    nc.tensor.matmul(
                        q_ps[:],
                        wqt_sb[cc][:],
                        xy_bf[cc][:, isl],
                        start=(cc == 0),
                        stop=(cc == CC - 1),
                    )
                nc.vector.tensor_scalar_add(
                    out=q_sb[:, isl], in0=q_ps[:], scalar1=bq_sb[:]
                )
                k_ps = ps_mm.tile([CQK, IB], F32, tag="mm", name="k_ps")
                for cc in range(CC):
                    nc.tensor.matmul(
                        k_ps[:],
                        wkt_sb[cc][:],
                        xv_bf[cc][:, isl],
                        start=(cc == 0),
                        stop=(cc == CC - 1),
                    )
                nc.vector.tensor_scalar_add(
                    out=k_sb[:, isl], in0=k_ps[:], scalar1=bk_sb[:]
                )

            # ---- Phase C: v^T projection (vT[j, c] = vit^T @ Wv^T) ----
            vt_sb = sg.tile([P, JT, C], BF)
            for jt in range(JT):
                jsl = slice(jt * P, (jt + 1) * P)
                vt_ps = ps_mm.tile([P, C], F32, tag="mm", name="vt_ps")
                for cc in range(CC):
                    nc.tensor.matmul(
                        vt_ps[:],
                        xv_bf[cc][:, jsl],
                        wvt_sb[cc][:],
                        start=(cc == 0),
                        stop=(cc == CC - 1),
                    )
                nc.vector.tensor_copy(vt_sb[:, jt, :], vt_ps[:])

            # ---- Phase D: attention + output, per i-block ----
            for ib in range(NIB):
                isl = slice(ib * IB, (ib + 1) * IB)
                u_ps = [
                    ps_u.tile([P, IB], F32, tag="u", name=f"u_ps{cc}")
                    for cc in range(CC)
                ]
                den_ps = ps_den.tile([P, IB], F32, tag="den", name="den_ps")
                for jt in range(JT):
                    jsl = slice(jt * P, (jt + 1) * P)
                    l_ps = ps_mm.tile([P, IB], F32, tag="mm", name="l_ps")
                    nc.tensor.matmul(
                        l_ps[:], k_sb[:, jsl], q_sb[:, isl], start=True, stop=True
                    )
                    p_sb = pp.tile([P, IB], BF, tag="p", name="p_sb")
                    nc.scalar.activation(
                        p_sb[:],
                        l_ps[:],
                        mybir.ActivationFunctionType.Exp,
                        bias=0.0,
                        scale=SCALE,
                    )
                    for cc in range(CC):
                        nc.tensor.matmul(
                            u_ps[cc][:],
                            vt_sb[:, jt, cc * P : (cc + 1) * P],
                            p_sb[:],
                            start=(jt == 0),
                            stop=(jt == JT - 1),
                        )
                    # denom[i] = sum_j P[j, i], replicated across partitions
                    nc.tensor.matmul(
                        den_ps[:],
                        ones_sb[:],
                        p_sb[:],
                        start=(jt == 0),
                        stop=(jt == JT - 1),
                    )

                r_sb = pr.tile([P, IB], F32, tag="r", name="r_sb")
                nc.vector.reciprocal(r_sb[:], den_ps[:])

                u_sb = []
                for cc in range(CC):
                    t = pu.tile([P, IB], BF, tag="usb", name=f"u_sb{cc}")
                    nc.vector.tensor_copy(t[:], u_ps[cc][:])
                    u_sb.append(t)

                for co in range(CC):
                    o_ps = ps_mm.tile([P, IB], F32, tag="mm", name="o_ps")
                    for cc in range(CC):
                        nc.tensor.matmul(
                            o_ps[:],
                            wot_sb[cc][:, co * P : (co + 1) * P],
                            u_sb[cc][:],
                            start=(cc == 0),
                            stop=(cc == CC - 1),
                        )
                    ot = po.tile([P, IB], F32, tag="ot", name="ot")
                    nc.vector.tensor_mul(ot[:], o_ps[:], r_sb[:])
                    nc.vector.tensor_add(ot[:], ot[:], xy[co][:, isl])
                    nc.sync.dma_start(out[co * P : (co + 1) * P, isl], ot[:])

    nc.compile()
    return nc


_NC_CACHE = {}


def _get_nc():
    if "nc" not in _NC_CACHE:
        _NC_CACHE["nc"] = build_nc()
    return _NC_CACHE["nc"]


def _prep_in_maps(inputs):
    yolo = np.ascontiguousarray(np.asarray(inputs["yolo_features"], np.float32))
    vit = np.ascontiguousarray(np.asarray(inputs["vit_features"], np.float32))
    Wq = np.asarray(inputs["Wq"], np.float32)
    bq = np.asarray(inputs["bq"], np.float32)
    Wk = np.asarray(inputs["Wk"], np.float32)
    bk = np.asarray(inputs["bk"], np.float32)
    Wv = np.asarray(inputs["Wv"], np.float32)
    bv = np.asarray(inputs["bv"], np.float32)
    Wo = np.asarray(inputs["Wo"], np.float32)
    bo = np.asarray(inputs["bo"], np.float32)

    wqt = np.ascontiguousarray(Wq.T).astype(BF16)
    wkt = np.ascontiguousarray(Wk.T).astype(BF16)
    wvt = np.ascontiguousarray(Wv.T).astype(BF16)
    wot = np.ascontiguousarray(Wo.T).astype(BF16)
    bq2 = np.ascontiguousarray(bq[:, None])
    bk2 = np.ascontiguousarray(bk[:, None])
    bop = np.ascontiguousarray((Wo @ bv + bo)[:, None].astype(np.float32))

    in_maps = []
    for b in range(B):
        in_maps.append(
            {
                "x_yolo": yolo[b].reshape(C, N),
                "x_vit": vit[b].reshape(C, N),
                "wqt": wqt,
                "wkt": wkt,
                "wvt": wvt,
                "wot": wot,
                "bq2": bq2,
                "bk2": bk2,
                "bop": bop,
            }
        )
    return in_maps


def run(inputs, trace=False):
    nc = _get_nc()
    in_maps = _prep_in_maps(inputs)
    res = run_bass_kernel_spmd(nc, in_maps, list(range(B)), trace=trace)
    out = np.stack([res.results[b]["out"] for b in range(B)], axis=0)
    return out.reshape(B, C, H, W).astype(np.float32), res


def kernel(**inputs):
    out, _ = run(inputs, trace=False)
    return out


# revision 2
# speedup vs baseline: 1.1110x; 1.1110x over previous
"""Trainium2 Bass kernel for CrossAttentionFusion.

Reference computation (per sample b):
    q = Wq @ yolo + bq            [32, N]    (N = 64*64 = 4096)
    k = Wk @ vit + bk             [32, N]
    v = Wv @ vit + bv             [256, N]
    A = softmax((q^T k) / sqrt(32), axis=j)         [N, N]
    out = yolo + Wo @ (v @ A^T) + bo                [256, N]

Sharding: data-parallel over batch B=8 across 8 cores; weights replicated.

Device algorithm (per core, one sample), bf16 matmuls with fp32 PSUM accum:
  - The output projection commutes into V: vo = (Wo @ Wv) @ vit gives
    out = yolo + (vo @ A^T) + (Wo @ bv + bo), so no O-projection on device.
  - vo^T[j, c] is produced directly by the projection (lhsT = vit chunk);
    nothing is ever transposed on device.
  - attnT[j, i] = k^T q is computed in [j, i] orientation so the softmax
    denominator and the A.V contraction both reduce over the PSUM partition
    axis. QK matmuls have K=32, so 4 j-tiles are packed into the 128-row PE
    array with tile_position row groups (q/k are built 4x-replicated across
    partition groups by col-packed projection matmuls).
  - P = exp(scale * attnT) without max subtraction (|logits| < 1 at this
    problem's scale; exp cannot overflow). One ACT instruction per 4 j-tiles
    ([128, 2048] across 4 PSUM banks) to amortize ACT fixed overhead.
  - denom[i] = sum_j P[j, i] via M=1 all-ones matmuls col-packed 4x into one
    PSUM bank (partials at partitions 0/32/64/96), merged + replicated to all
    128 partitions by a single masked matmul (sel4), then reciprocal.
  - U[c, i] = sum_j voT[j, c] P[j, i] accumulates unnormalized; the epilogue
    applies U * (1/denom) + yolo + (Wo@bv + bo) in fp32.
"""

import sys

sys.path.insert(0, "/opt/trn_rl_repo")

import numpy as np
import ml_dtypes

import concourse.bass as bass
import concourse.tile as tile
from concourse import bacc, mybir
from concourse.bass_utils import run_bass_kernel_spmd

BF16 = ml_dtypes.bfloat16
F32 = mybir.dt.float32
BF = mybir.dt.bfloat16

B, C, H, W = 8, 256, 64, 64
N = H * W            # 4096
CQK = C // 8         # 32
P = 128              # partitions
IB = 512             # i-block (one PSUM bank of fp32)
NIB = N // IB        # 8
JT = N // P          # 32 j-tiles
JG = JT // 4         # 8 groups of 4 j-tiles
CC = C // P          # 2 channel chunks
SCALE = 1.0 / float(np.sqrt(np.float32(CQK)))


def build_nc():
    nc = bacc.Bacc("TRN2", target_bir_lowering=False, debug=False)

    x_yolo = nc.dram_tensor("x_yolo", [C, N], F32, kind="ExternalInput")
    x_vit = nc.dram_tensor("x_vit", [C, N], F32, kind="ExternalInput")
    wqt = nc.dram_tensor("wqt", [C, CQK], BF, kind="ExternalInput")
    wkt = nc.dram_tensor("wkt", [C, CQK], BF, kind="ExternalInput")
    wvo = nc.dram_tensor("wvo", [C, C], BF, kind="ExternalInput")  # (Wo@Wv)^T
    bq4 = nc.dram_tensor("bq4", [P, 1], F32, kind="ExternalInput")  # tile(bq,4)
    bk4 = nc.dram_tensor("bk4", [P, 1], F32, kind="ExternalInput")
    bop = nc.dram_tensor("bop", [C, 1], F32, kind="ExternalInput")  # Wo@bv+bo
    sel4 = nc.dram_tensor("sel4", [P, P], BF, kind="ExternalInput")
    out = nc.dram_tensor("out", [C, N], F32, kind="ExternalOutput")

    with tile.TileContext(nc) as tc:
        with (
            tc.tile_pool(name="sg", bufs=1) as sg,
            tc.tile_pool(name="pxv", bufs=2) as pxv,
            tc.tile_pool(name="pp4", bufs=16) as pp4,
            tc.tile_pool(name="pr", bufs=2) as pr,
            tc.tile_pool(name="pot", bufs=4) as pot,
            tc.tile_pool(name="ps_l", bufs=1, space="PSUM") as ps_l,
            tc.tile_pool(name="ps_u", bufs=1, space="PSUM") as ps_u,
            tc.tile_pool(name="ps_den", bufs=1, space="PSUM") as ps_den,
            tc.tile_pool(name="ps_misc", bufs=2, space="PSUM") as ps_misc,
        ):
            # ---- Phase A: loads, casts, residual-with-bias precompute ----
            wqt_sb = []
            wkt_sb = []
            wvo_sb = []
            for cc in range(CC):
                csl = slice(cc * P, (cc + 1) * P)
                t = sg.tile([P, CQK], BF, name=f"wqt{cc}")
                nc.sync.dma_start(t[:], wqt[csl, :])
                wqt_sb.append(t)
                t = sg.tile([P, CQK], BF, name=f"wkt{cc}")
                nc.sync.dma_start(t[:], wkt[csl, :])
                wkt_sb.append(t)
                t = sg.tile([P, C], BF, name=f"wvo{cc}")
                nc.sync.dma_start(t[:], wvo[csl, :])
                wvo_sb.append(t)

            bq_sb = sg.tile([P, 1], F32)
            nc.sync.dma_start(bq_sb[:], bq4[:])
            bk_sb = sg.tile([P, 1], F32)
            nc.sync.dma_start(bk_sb[:], bk4[:])
            sel4_sb = sg.tile([P, P], BF)
            nc.sync.dma_start(sel4_sb[:], sel4[:])
            bop_sb = []
            for cc in range(CC):
                t = sg.tile([P, 1], F32, name=f"bop{cc}")
                nc.sync.dma_start(t[:], bop[cc * P : (cc + 1) * P, :])
                bop_sb.append(t)

            ones1 = sg.tile([P, 1], BF)
            nc.vector.memset(ones1[:], 1.0)
            den4_sb = sg.tile([P, IB], BF)
            nc.vector.memset(den4_sb[:], 0.0)

            # yolo: keep fp32 (becomes yb = yolo + bop), plus bf16 copy for q
            yb = []
            xy_bf = []
            for cc in range(CC):
                csl = slice(cc * P, (cc + 1) * P)
                t = sg.tile([P, N], F32, name=f"yb{cc}")
                nc.sync.dma_start(t[:], x_yolo[csl, :])
                yb.append(t)
                tb = sg.tile([P, N], BF, name=f"xybf{cc}")
                nc.vector.tensor_copy(tb[:], t[:])
                xy_bf.append(tb)
                nc.vector.tensor_scalar_add(out=t[:], in0=t[:], scalar1=bop_sb[cc][:])

            # vit: fp32 staging is transient (pool-rotated), keep bf16
            xv_bf = []
            for cc in range(CC):
                csl = slice(cc * P, (cc + 1) * P)
                xvf = pxv.tile([P, N], F32, tag="xvf", name="xvf")
                nc.sync.dma_start(xvf[:], x_vit[csl, :])
                tb = sg.tile([P, N], BF, name=f"xvbf{cc}")
                nc.vector.tensor_copy(tb[:], xvf[:])
                xv_bf.append(tb)

            # ---- Phase B: q/k projections, built 4x-replicated across
            # partition groups via col-packed matmuls ----
            q_sb = sg.tile([P, N], BF)
            k_sb = sg.tile([P, N], BF)
            for ic in range(NIB):
                isl = slice(ic * IB, (ic + 1) * IB)
                for (dst, wt, bias, src) in (
                    (q_sb, wqt_sb, bq_sb, xy_bf),
                    (k_sb, wkt_sb, bk_sb, xv_bf),
                ):
                    prj = ps_misc.tile([P, IB], F32, tag="misc", name="prj")
                    for g in range(4):
                        for cc in range(CC):
                            nc.tensor.matmul(
                                prj[32 * g : 32 * (g + 1), :],
                                wt[cc][:],
                                src[cc][:, isl],
                                start=(cc == 0),
                                stop=(cc == CC - 1),
                                tile_position=(0, 32 * g),
                            )
                    nc.vector.tensor_scalar_add(
                        out=dst[:, isl], in0=prj[:], scalar1=bias[:]
                    )

            # ---- Phase C: vo^T projection (voT[j, c] = vit^T @ (Wo Wv)^T) ----
            vo_sb = sg.tile([P, JT, C], BF)
            for jt in range(JT):
                jsl = slice(jt * P, (jt + 1) * P)
                vo_ps = ps_misc.tile([P, C], F32, tag="misc", name="vo_ps")
                for cc in range(CC):
                    nc.tensor.matmul(
                        vo_ps[:],
                        xv_bf[cc][:, jsl],
                        wvo_sb[cc][:],
                        start=(cc == 0),
                        stop=(cc == CC - 1),
                    )
                nc.vector.tensor_copy(vo_sb[:, jt, :], vo_ps[:])

            # ---- Phase D: attention per i-block ----
            for ib in range(NIB):
                isl = slice(ib * IB, (ib + 1) * IB)

                # D1: QK (4x row-packed) + exp + denom partials (4x col-packed)
                den_ps = ps_den.tile([P, IB], F32, tag="den", name="den_ps")
                p4s = []
                for G in range(JG):
                    l_ps = ps_l.tile([P, 4, IB], F32, tag="l", name="l_ps")
                    for g in range(4):
                        jt = 4 * G + g
                        gsl = slice(32 * g, 32 * (g + 1))
                        nc.tensor.matmul(
                            l_ps[:, g, :],
                            k_sb[gsl, jt * P : (jt + 1) * P],
                            q_sb[gsl, isl],
                            start=True,
                            stop=True,
                            tile_position=(32 * g, 0),
                        )
                    p4 = pp4.tile([P, 4, IB], BF, tag="p4", name="p4")
                    nc.scalar.activation(
                        p4[:],
                        l_ps[:],
                        mybir.ActivationFunctionType.Exp,
                        bias=0.0,
                        scale=SCALE,
                    )
                    p4s.append(p4)
                    for g in range(4):
                        nc.tensor.matmul(
                            den_ps[32 * g : 32 * g + 1, :],
                            ones1[:],
                            p4[:, g, :],
                            start=(G == 0),
                            stop=(G == JG - 1),
                            tile_position=(0, 32 * g),
                        )

                # denom: merge 4 partials + replicate to 128 partitions
                for g in range(4):
                    nc.vector.tensor_copy(
                        den4_sb[32 * g : 32 * g + 1, :],
                        den_ps[32 * g : 32 * g + 1, :],
                    )
                rep_ps = ps_misc.tile([P, IB], F32, tag="misc", name="rep_ps")
                nc.tensor.matmul(rep_ps[:], sel4_sb[:], den4_sb[:], start=True, stop=True)
                r_sb = pr.tile([P, IB], F32, tag="r", name="r_sb")
                nc.vector.reciprocal(r_sb[:], rep_ps[:])

                # D2: U accumulation + epilogue, one channel chunk at a time
                for cc in range(CC):
                    u_ps = ps_u.tile([P, IB], F32, tag="u", name="u_ps")
                    for G in range(JG):
                        for g in range(4):
                            jt = 4 * G + g
                            nc.tensor.matmul(
                                u_ps[:],
                                vo_sb[:, jt, cc * P : (cc + 1) * P],
                                p4s[G][:, g, :],
                                start=(G == 0 and g == 0),
                                stop=(G == JG - 1 and g == 3),
                            )
                    ot = pot.tile([P, IB], F32, tag="ot", name="ot")
                    nc.vector.tensor_mul(ot[:], u_ps[:], r_sb[:])
                    nc.vector.tensor_add(ot[:], ot[:], yb[cc][:, isl])
                    nc.sync.dma_start(out[cc * P : (cc + 1) * P, isl], ot[:])

    nc.compile()
    return nc


_NC_CACHE = {}


def _get_nc():
    if "nc" not in _NC_CACHE:
        _NC_CACHE["nc"] = build_nc()
    return _NC_CACHE["nc"]


def _prep_in_maps(inputs):
    yolo = np.ascontiguousarray(np.asarray(inputs["yolo_features"], np.float32))
    vit = np.ascontiguousarray(np.asarray(inputs["vit_features"], np.float32))
    Wq = np.asarray(inputs["Wq"], np.float32)
    bq = np.asarray(inputs["bq"], np.float32)
    Wk = np.asarray(inputs["Wk"], np.float32)
    bk = np.asarray(inputs["bk"], np.float32)
    Wv = np.asarray(inputs["Wv"], np.float32)
    bv = np.asarray(inputs["bv"], np.float32)
    Wo = np.asarray(inputs["Wo"], np.float32)
    bo = np.asarray(inputs["bo"], np.float32)

    wqt = np.ascontiguousarray(Wq.T).astype(BF16)
    wkt = np.ascontiguousarray(Wk.T).astype(BF16)
    wvo = np.ascontiguousarray((Wo @ Wv).T).astype(BF16)
    bq4 = np.ascontiguousarray(np.tile(bq, 4)[:, None].astype(np.float32))
    bk4 = np.ascontiguousarray(np.tile(bk, 4)[:, None].astype(np.float32))
    bop = np.ascontiguousarray((Wo @ bv + bo)[:, None].astype(np.float32))
    sel4 = np.zeros((P, P), dtype=BF16)
    sel4[[0, 32, 64, 96], :] = 1.0

    in_maps = []
    for b in range(B):
        in_maps.append(
            {
                "x_yolo": yolo[b].reshape(C, N),
                "x_vit": vit[b].reshape(C, N),
                "wqt": wqt,
                "wkt": wkt,
                "wvo": wvo,
                "bq4": bq4,
                "bk4": bk4,
                "bop": bop,
                "sel4": sel4,
            }
        )
    return in_maps


def run(inputs, trace=False):
    nc = _get_nc()
    in_maps = _prep_in_maps(inputs)
    res = run_bass_kernel_spmd(nc, in_maps, list(range(B)), trace=trace)
    out = np.stack([res.results[b]["out"] for b in range(B)], axis=0)
    return out.reshape(B, C, H, W).astype(np.float32), res


def kernel(**inputs):
    out, _ = run(inputs, trace=False)
    return out


# revision 3
# speedup vs baseline: 1.2946x; 1.1652x over previous
"""Trainium2 Bass kernel for CrossAttentionFusion.

Reference computation (per sample b):
    q = Wq @ yolo + bq            [32, N]    (N = 64*64 = 4096)
    k = Wk @ vit + bk             [32, N]
    v = Wv @ vit + bv             [256, N]
    A = softmax((q^T k) / sqrt(32), axis=j)         [N, N]
    out = yolo + Wo @ (v @ A^T) + bo                [256, N]

Sharding: data-parallel over batch B=8 across 8 cores; weights replicated.

Device algorithm (per core, one sample), bf16 matmuls with fp32 PSUM accum:
  - The output projection commutes into V: vo = (Wo @ Wv) @ vit gives
    out = yolo + (vo @ A^T) + (Wo @ bv + bo), so no O-projection on device.
  - vo^T[j, c] is produced directly by the projection (lhsT = vit chunk);
    nothing is ever transposed on device.
  - attnT[j, i] = k^T q is computed in [j, i] orientation so the softmax
    denominator and the A.V contraction both reduce over the PSUM partition
    axis. QK matmuls have K=32, so 4 j-tiles are packed into the 128-row PE
    array with tile_position row groups (q/k are built 4x-replicated across
    partition groups by col-packed projection matmuls).
  - P = exp(scale * attnT) without max subtraction (|logits| < 1 at this
    problem's scale; exp cannot overflow). One ACT instruction per 4 j-tiles
    ([128, 2048] across 4 PSUM banks) to amortize ACT fixed overhead.
  - denom[i] = sum_j P[j, i] via M=1 all-ones matmuls col-packed 4x into one
    PSUM bank (partials at partitions 0/32/64/96), merged + replicated to all
    128 partitions by a single masked matmul (sel4), then reciprocal.
  - U[c, i] = sum_j voT[j, c] P[j, i] accumulates unnormalized; the epilogue
    applies U * (1/denom) + yolo + (Wo@bv + bo) in fp32.
"""

import sys

sys.path.insert(0, "/opt/trn_rl_repo")

import numpy as np
import ml_dtypes

import concourse.bass as bass
import concourse.tile as tile
from concourse import bacc, mybir
from concourse.bass_utils import run_bass_kernel_spmd

BF16 = ml_dtypes.bfloat16
F32 = mybir.dt.float32
BF = mybir.dt.bfloat16

B, C, H, W = 8, 256, 64, 64
N = H * W            # 4096
CQK = C // 8         # 32
P = 128              # partitions
IB = 512             # i-block (one PSUM bank of fp32)
NIB = N // IB        # 8
JT = N // P          # 32 j-tiles
JG = JT // 4         # 8 groups of 4 j-tiles
CC = C // P          # 2 channel chunks
SCALE = 1.0 / float(np.sqrt(np.float32(CQK)))


def build_nc():
    nc = bacc.Bacc("TRN2", target_bir_lowering=False, debug=False)

    x_yolo = nc.dram_tensor("x_yolo", [C, N], F32, kind="ExternalInput")
    x_vit = nc.dram_tensor("x_vit", [C, N], F32, kind="ExternalInput")
    wqt = nc.dram_tensor("wqt", [C, CQK], BF, kind="ExternalInput")
    wkt = nc.dram_tensor("wkt", [C, CQK], BF, kind="ExternalInput")
    wvo = nc.dram_tensor("wvo", [C, C], BF, kind="ExternalInput")  # (Wo@Wv)^T
    bq4 = nc.dram_tensor("bq4", [P, 1], F32, kind="ExternalInput")  # tile(bq,4)
    bk4 = nc.dram_tensor("bk4", [P, 1], F32, kind="ExternalInput")
    bop = nc.dram_tensor("bop", [C, 1], F32, kind="ExternalInput")  # Wo@bv+bo
    sel4 = nc.dram_tensor("sel4", [P, P], BF, kind="ExternalInput")
    out = nc.dram_tensor("out", [C, N], F32, kind="ExternalOutput")

    with tile.TileContext(nc) as tc:
        with (
            tc.tile_pool(name="sg", bufs=1) as sg,
            tc.tile_pool(name="pxv", bufs=2) as pxv,
            tc.tile_pool(name="pp4", bufs=16) as pp4,
            tc.tile_pool(name="pr", bufs=2) as pr,
            tc.tile_pool(name="pot", bufs=4) as pot,
            tc.tile_pool(name="ps_l", bufs=1, space="PSUM") as ps_l,
            tc.tile_pool(name="ps_u", bufs=1, space="PSUM") as ps_u,
            tc.tile_pool(name="ps_den", bufs=1, space="PSUM") as ps_den,
            tc.tile_pool(name="ps_misc", bufs=2, space="PSUM") as ps_misc,
        ):
            # ---- Phase A: loads, casts, residual-with-bias precompute ----
            wqt_sb = []
            wkt_sb = []
            wvo_sb = []
            for cc in range(CC):
                csl = slice(cc * P, (cc + 1) * P)
                t = sg.tile([P, CQK], BF, name=f"wqt{cc}")
                nc.sync.dma_start(t[:], wqt[csl, :])
                wqt_sb.append(t)
                t = sg.tile([P, CQK], BF, name=f"wkt{cc}")
                nc.sync.dma_start(t[:], wkt[csl, :])
                wkt_sb.append(t)
                t = sg.tile([P, C], BF, name=f"wvo{cc}")
                nc.sync.dma_start(t[:], wvo[csl, :])
                wvo_sb.append(t)

            bq_sb = sg.tile([P, 1], F32)
            nc.sync.dma_start(bq_sb[:], bq4[:])
            bk_sb = sg.tile([P, 1], F32)
            nc.sync.dma_start(bk_sb[:], bk4[:])
            sel4_sb = sg.tile([P, P], BF)
            nc.sync.dma_start(sel4_sb[:], sel4[:])
            bop_sb = []
            for cc in range(CC):
                t = sg.tile([P, 1], F32, name=f"bop{cc}")
                nc.sync.dma_start(t[:], bop[cc * P : (cc + 1) * P, :])
                bop_sb.append(t)

            ones1 = sg.tile([P, 1], BF)
            nc.vector.memset(ones1[:], 1.0)
            den4_sb = sg.tile([P, IB], BF)
            nc.vector.memset(den4_sb[:], 0.0)

            # yolo: keep fp32 (becomes yb = yolo + bop), plus bf16 copy for q
            # (casts on DVE; vit casts on ACT so the two streams overlap)
            yb = []
            xy_bf = []
            for cc in range(CC):
                csl = slice(cc * P, (cc + 1) * P)
                t = sg.tile([P, N], F32, name=f"yb{cc}")
                nc.sync.dma_start(t[:], x_yolo[csl, :])
                yb.append(t)
                tb = sg.tile([P, N], BF, name=f"xybf{cc}")
                nc.vector.tensor_copy(tb[:], t[:])
                xy_bf.append(tb)
                nc.vector.tensor_scalar_add(out=t[:], in0=t[:], scalar1=bop_sb[cc][:])

            # vit: fp32 staging is transient (pool-rotated), keep bf16
            xv_bf = []
            for cc in range(CC):
                csl = slice(cc * P, (cc + 1) * P)
                xvf = pxv.tile([P, N], F32, tag="xvf", name="xvf")
                nc.sync.dma_start(xvf[:], x_vit[csl, :])
                tb = sg.tile([P, N], BF, name=f"xvbf{cc}")
                nc.scalar.copy(tb[:], xvf[:])
                xv_bf.append(tb)

            q_sb = sg.tile([P, N], BF)
            k_sb = sg.tile([P, N], BF)
            vo_sb = sg.tile([P, JT, C], BF)

            def emit_qk_proj(dst, wt, bias, src, ic):
                isl = slice(ic * IB, (ic + 1) * IB)
                prj = ps_misc.tile([P, IB], F32, tag="misc", name="prj")
                for g in range(4):
                    for cc in range(CC):
                        nc.tensor.matmul(
                            prj[32 * g : 32 * (g + 1), :],
                            wt[cc][:],
                            src[cc][:, isl],
                            start=(cc == 0),
                            stop=(cc == CC - 1),
                            tile_position=(0, 32 * g),
                        )
                nc.vector.tensor_scalar_add(out=dst[:, isl], in0=prj[:], scalar1=bias[:])

            def emit_vo_proj(jt):
                jsl = slice(jt * P, (jt + 1) * P)
                vo_ps = ps_misc.tile([P, C], F32, tag="misc", name="vo_ps")
                for cc in range(CC):
                    nc.tensor.matmul(
                        vo_ps[:],
                        xv_bf[cc][:, jsl],
                        wvo_sb[cc][:],
                        start=(cc == 0),
                        stop=(cc == CC - 1),
                    )
                nc.vector.tensor_copy(vo_sb[:, jt, :], vo_ps[:])

            # D1 group: QK (4x row-packed) + exp + denom partials (col-packed)
            def emit_d1_group(ib, G, den_ps, p4s):
                isl = slice(ib * IB, (ib + 1) * IB)
                l_ps = ps_l.tile([P, 4, IB], F32, tag="l", name="l_ps")
                for g in range(4):
                    jt = 4 * G + g
                    gsl = slice(32 * g, 32 * (g + 1))
                    nc.tensor.matmul(
                        l_ps[:, g, :],
                        k_sb[gsl, jt * P : (jt + 1) * P],
                        q_sb[gsl, isl],
                        start=True,
                        stop=True,
                        tile_position=(32 * g, 0),
                    )
                p4 = pp4.tile([P, 4, IB], BF, tag="p4", name="p4")
                nc.scalar.activation(
                    p4[:],
                    l_ps[:],
                    mybir.ActivationFunctionType.Exp,
                    bias=0.0,
                    scale=SCALE,
                )
                p4s.append(p4)
                for g in range(4):
                    nc.tensor.matmul(
                        den_ps[32 * g : 32 * g + 1, :],
                        ones1[:],
                        p4[:, g, :],
                        start=(G == 0),
                        stop=(G == JG - 1),
                        tile_position=(0, 32 * g),
                    )

            # ---- Prologue: q/k/vo projections interleaved with D1(ib=0) ----
            # D1(0, G) needs k columns of its own j-tiles (= k-proj ic G) and
            # q columns 0:512 (= q-proj ic 0), so the stagger below keeps PE,
            # ACT and DVE all busy from the start.
            den_ps_cur = ps_den.tile([P, IB], F32, tag="den", name="den_ps")
            p4s_cur = []
            for G in range(JG):
                emit_qk_proj(k_sb, wkt_sb, bk_sb, xv_bf, G)
                emit_qk_proj(q_sb, wqt_sb, bq_sb, xy_bf, G)
                for g in range(4):
                    emit_vo_proj(4 * G + g)
                emit_d1_group(0, G, den_ps_cur, p4s_cur)

            # ---- Main loop: D2(ib) with D1(ib+1) interleaved ----
            for ib in range(NIB):
                isl = slice(ib * IB, (ib + 1) * IB)

                # denom for this block: merge 4 partials + replicate + recip
                den_ps, p4s = den_ps_cur, p4s_cur
                for g in range(4):
                    nc.vector.tensor_copy(
                        den4_sb[32 * g : 32 * g + 1, :],
                        den_ps[32 * g : 32 * g + 1, :],
                    )
                rep_ps = ps_misc.tile([P, IB], F32, tag="misc", name="rep_ps")
                nc.tensor.matmul(
                    rep_ps[:], sel4_sb[:], den4_sb[:], start=True, stop=True
                )
                r_sb = pr.tile([P, IB], F32, tag="r", name="r_sb")
                nc.vector.reciprocal(r_sb[:], rep_ps[:])

                if ib + 1 < NIB:
                    den_ps_cur = ps_den.tile([P, IB], F32, tag="den", name="den_ps")
                    p4s_cur = []
                    d1_next = [(ib + 1, G) for G in range(JG)]
                else:
                    d1_next = []

                step = 0
                for cc in range(CC):
                    u_ps = ps_u.tile([P, IB], F32, tag="u", name="u_ps")
                    for G in range(JG):
                        for g in range(4):
                            jt = 4 * G + g
                            nc.tensor.matmul(
                                u_ps[:],
                                vo_sb[:, jt, cc * P : (cc + 1) * P],
                                p4s[G][:, g, :],
                                start=(G == 0 and g == 0),
                                stop=(G == JG - 1 and g == 3),
                            )
                        if step % 2 == 1 and d1_next:
                            nib, nG = d1_next.pop(0)
                            emit_d1_group(nib, nG, den_ps_cur, p4s_cur)
                        step += 1
                    ot = pot.tile([P, IB], F32, tag="ot", name="ot")
                    nc.vector.tensor_mul(ot[:], u_ps[:], r_sb[:])
                    nc.vector.tensor_add(ot[:], ot[:], yb[cc][:, isl])
                    nc.sync.dma_start(out[cc * P : (cc + 1) * P, isl], ot[:])

    nc.compile()
    return nc


_NC_CACHE = {}


def _get_nc():
    if "nc" not in _NC_CACHE:
        _NC_CACHE["nc"] = build_nc()
    return _NC_CACHE["nc"]


def _prep_in_maps(inputs):
    yolo = np.ascontiguousarray(np.asarray(inputs["yolo_features"], np.float32))
    vit = np.ascontiguousarray(np.asarray(inputs["vit_features"], np.float32))
    Wq = np.asarray(inputs["Wq"], np.float32)
    bq = np.asarray(inputs["bq"], np.float32)
    Wk = np.asarray(inputs["Wk"], np.float32)
    bk = np.asarray(inputs["bk"], np.float32)
    Wv = np.asarray(inputs["Wv"], np.float32)
    bv = np.asarray(inputs["bv"], np.float32)
    Wo = np.asarray(inputs["Wo"], np.float32)
    bo = np.asarray(inputs["bo"], np.float32)

    wqt = np.ascontiguousarray(Wq.T).astype(BF16)
    wkt = np.ascontiguousarray(Wk.T).astype(BF16)
    wvo = np.ascontiguousarray((Wo @ Wv).T).astype(BF16)
    bq4 = np.ascontiguousarray(np.tile(bq, 4)[:, None].astype(np.float32))
    bk4 = np.ascontiguousarray(np.tile(bk, 4)[:, None].astype(np.float32))
    bop = np.ascontiguousarray((Wo @ bv + bo)[:, None].astype(np.float32))
    sel4 = np.zeros((P, P), dtype=BF16)
    sel4[[0, 32, 64, 96], :] = 1.0

    in_maps = []
    for b in range(B):
        in_maps.append(
            {
                "x_yolo": yolo[b].reshape(C, N),
                "x_vit": vit[b].reshape(C, N),
                "wqt": wqt,
                "wkt": wkt,
                "wvo": wvo,
                "bq4": bq4,
                "bk4": bk4,
                "bop": bop,
                "sel4": sel4,
            }
        )
    return in_maps


def run(inputs, trace=False):
    nc = _get_nc()
    in_maps = _prep_in_maps(inputs)
    res = run_bass_kernel_spmd(nc, in_maps, list(range(B)), trace=trace)
    out = np.stack([res.results[b]["out"] for b in range(B)], axis=0)
    return out.reshape(B, C, H, W).astype(np.float32), res


def kernel(**inputs):
    out, _ = run(inputs, trace=False)
    return out


# revision 5
# speedup vs baseline: 1.3555x; 1.0471x over previous
"""Trainium2 Bass kernel for CrossAttentionFusion.

Reference computation (per sample b):
    q = Wq @ yolo + bq            [32, N]    (N = 64*64 = 4096)
    k = Wk @ vit + bk             [32, N]
    v = Wv @ vit + bv             [256, N]
    A = softmax((q^T k) / sqrt(32), axis=j)         [N, N]
    out = yolo + Wo @ (v @ A^T) + bo                [256, N]

Sharding: data-parallel over batch B=8 across 8 cores; weights replicated.

Device algorithm (per core, one sample), bf16 matmuls with fp32 PSUM accum:
  - The output projection commutes into V: vo = (Wo @ Wv) @ vit gives
    out = yolo + (vo @ A^T) + (Wo @ bv + bo), so no O-projection on device.
  - vo^T[j, c] is produced directly by the projection (lhsT = vit chunk);
    nothing is ever transposed on device.
  - attnT[j, i] = k^T q is computed in [j, i] orientation so the softmax
    denominator and the A.V contraction both reduce over the PSUM partition
    axis. QK matmuls have K=32, so 4 j-tiles are packed into the 128-row PE
    array with tile_position row groups (q/k are built 4x-replicated across
    partition groups by col-packed projection matmuls).
  - P = exp(scale * attnT) without max subtraction (|logits| < 1 at this
    problem's scale; exp cannot overflow). One ACT instruction per 4 j-tiles
    ([128, 2048] across 4 PSUM banks) to amortize ACT fixed overhead.
  - denom[i] = sum_j P[j, i] via M=1 all-ones matmuls col-packed 4x into one
    PSUM bank (partials at partitions 0/32/64/96), merged + replicated to all
    128 partitions by a single masked matmul (sel4), then reciprocal.
  - U[c, i] = sum_j voT[j, c] P[j, i] accumulates unnormalized; the epilogue
    applies U * (1/denom) + yolo + (Wo@bv + bo) in fp32.
"""

import sys

sys.path.insert(0, "/opt/trn_rl_repo")

import numpy as np
import ml_dtypes

import concourse.bass as bass
import concourse.tile as tile
from concourse import bacc, mybir
from concourse.bass_utils import run_bass_kernel_spmd

BF16 = ml_dtypes.bfloat16
F32 = mybir.dt.float32
BF = mybir.dt.bfloat16

B, C, H, W = 8, 256, 64, 64
N = H * W            # 4096
CQK = C // 8         # 32
P = 128              # partitions
IB = 512             # i-block (one PSUM bank of fp32)
NIB = N // IB        # 8
JT = N // P          # 32 j-tiles
JG = JT // 4         # 8 groups of 4 j-tiles
CC = C // P          # 2 channel chunks
SCALE = 1.0 / float(np.sqrt(np.float32(CQK)))


def build_nc():
    nc = bacc.Bacc("TRN2", target_bir_lowering=False, debug=False)

    x_yolo = nc.dram_tensor("x_yolo", [C, N], F32, kind="ExternalInput")
    x_vit = nc.dram_tensor("x_vit", [C, N], F32, kind="ExternalInput")
    wqt = nc.dram_tensor("wqt", [C, CQK], BF, kind="ExternalInput")
    wkt = nc.dram_tensor("wkt", [C, CQK], BF, kind="ExternalInput")
    wvo = nc.dram_tensor("wvo", [C, C], BF, kind="ExternalInput")  # (Wo@Wv)^T
    bq4 = nc.dram_tensor("bq4", [P, 1], F32, kind="ExternalInput")  # tile(bq,4)
    bk4 = nc.dram_tensor("bk4", [P, 1], F32, kind="ExternalInput")
    bop = nc.dram_tensor("bop", [C, 1], F32, kind="ExternalInput")  # Wo@bv+bo
    sel4 = nc.dram_tensor("sel4", [P, P], BF, kind="ExternalInput")
    out = nc.dram_tensor("out", [C, N], F32, kind="ExternalOutput")

    with tile.TileContext(nc) as tc:
        with (
            tc.tile_pool(name="sg", bufs=1) as sg,
            tc.tile_pool(name="pxv", bufs=2) as pxv,
            tc.tile_pool(name="pp4", bufs=16) as pp4,
            tc.tile_pool(name="pr", bufs=2) as pr,
            tc.tile_pool(name="pot", bufs=4) as pot,
            tc.tile_pool(name="ps_l", bufs=1, space="PSUM") as ps_l,
            tc.tile_pool(name="ps_u", bufs=1, space="PSUM") as ps_u,
            tc.tile_pool(name="ps_den", bufs=1, space="PSUM") as ps_den,
            tc.tile_pool(name="ps_misc", bufs=2, space="PSUM") as ps_misc,
        ):
            # ---- Phase A: loads, casts, residual-with-bias precompute ----
            wqt_sb = []
            wkt_sb = []
            wvo_sb = []
            for cc in range(CC):
                csl = slice(cc * P, (cc + 1) * P)
                t = sg.tile([P, CQK], BF, name=f"wqt{cc}")
                nc.sync.dma_start(t[:], wqt[csl, :])
                wqt_sb.append(t)
                t = sg.tile([P, CQK], BF, name=f"wkt{cc}")
                nc.sync.dma_start(t[:], wkt[csl, :])
                wkt_sb.append(t)
                t = sg.tile([P, C], BF, name=f"wvo{cc}")
                nc.sync.dma_start(t[:], wvo[csl, :])
                wvo_sb.append(t)

            bq_sb = sg.tile([P, 1], F32)
            nc.sync.dma_start(bq_sb[:], bq4[:])
            bk_sb = sg.tile([P, 1], F32)
            nc.sync.dma_start(bk_sb[:], bk4[:])
            sel4_sb = sg.tile([P, P], BF)
            nc.sync.dma_start(sel4_sb[:], sel4[:])
            bop_sb = []
            for cc in range(CC):
                t = sg.tile([P, 1], F32, name=f"bop{cc}")
                nc.sync.dma_start(t[:], bop[cc * P : (cc + 1) * P, :])
                bop_sb.append(t)

            ones1 = sg.tile([P, 1], BF)
            nc.vector.memset(ones1[:], 1.0)
            den4_sb = sg.tile([P, IB], BF)
            nc.vector.memset(den4_sb[:], 0.0)

            # Inputs are loaded in column chunks so casts/compute start while
            # later chunks are still in flight. vit casts run on ACT, yolo
            # casts on DVE (parallel streams). yolo fp32 stays resident and
            # becomes yb = yolo + bop (the epilogue addend).
            XCH = 1024
            yb = [sg.tile([P, N], F32, name=f"yb{cc}") for cc in range(CC)]
            xy_bf = [sg.tile([P, N], BF, name=f"xybf{cc}") for cc in range(CC)]
            xv_bf = [sg.tile([P, N], BF, name=f"xvbf{cc}") for cc in range(CC)]
            for x0 in range(0, N, XCH):
                xsl = slice(x0, x0 + XCH)
                for cc in range(CC):
                    csl = slice(cc * P, (cc + 1) * P)
                    xvf = pxv.tile([P, XCH], F32, tag="xvf", name="xvf")
                    nc.sync.dma_start(xvf[:], x_vit[csl, xsl])
                    nc.scalar.copy(xv_bf[cc][:, xsl], xvf[:])
                for cc in range(CC):
                    csl = slice(cc * P, (cc + 1) * P)
                    nc.sync.dma_start(yb[cc][:, xsl], x_yolo[csl, xsl])
                    nc.vector.tensor_copy(xy_bf[cc][:, xsl], yb[cc][:, xsl])
            for cc in range(CC):
                nc.vector.tensor_scalar_add(
                    out=yb[cc][:], in0=yb[cc][:], scalar1=bop_sb[cc][:]
                )

            q_sb = sg.tile([P, N], BF)
            k_sb = sg.tile([P, N], BF)
            vo_sb = sg.tile([P, JT, C], BF)

            def emit_qk_proj(dst, wt, bias, src, ic):
                isl = slice(ic * IB, (ic + 1) * IB)
                prj = ps_misc.tile([P, IB], F32, tag="misc", name="prj")
                for g in range(4):
                    for cc in range(CC):
                        nc.tensor.matmul(
                            prj[32 * g : 32 * (g + 1), :],
                            wt[cc][:],
                            src[cc][:, isl],
                            start=(cc == 0),
                            stop=(cc == CC - 1),
                            tile_position=(0, 32 * g),
                        )
                nc.vector.tensor_scalar_add(out=dst[:, isl], in0=prj[:], scalar1=bias[:])

            def emit_vo_proj(jt):
                jsl = slice(jt * P, (jt + 1) * P)
                vo_ps = ps_misc.tile([P, C], F32, tag="misc", name="vo_ps")
                for cc in range(CC):
                    nc.tensor.matmul(
                        vo_ps[:],
                        xv_bf[cc][:, jsl],
                        wvo_sb[cc][:],
                        start=(cc == 0),
                        stop=(cc == CC - 1),
                    )
                nc.vector.tensor_copy(vo_sb[:, jt, :], vo_ps[:])

            # D1 group: QK (4x row-packed) + exp + denom partials (col-packed)
            def emit_d1_group(ib, G, den_ps, p4s):
                isl = slice(ib * IB, (ib + 1) * IB)
                l_ps = ps_l.tile([P, 4, IB], F32, tag="l", name="l_ps")
                for g in range(4):
                    jt = 4 * G + g
                    gsl = slice(32 * g, 32 * (g + 1))
                    nc.tensor.matmul(
                        l_ps[:, g, :],
                        k_sb[gsl, jt * P : (jt + 1) * P],
                        q_sb[gsl, isl],
                        start=True,
                        stop=True,
                        tile_position=(32 * g, 0),
                    )
                p4 = pp4.tile([P, 4, IB], BF, tag="p4", name="p4")
                nc.scalar.activation(
                    p4[:],
                    l_ps[:],
                    mybir.ActivationFunctionType.Exp,
                    bias=0.0,
                    scale=SCALE,
                )
                p4s.append(p4)
                for g in range(4):
                    nc.tensor.matmul(
                        den_ps[32 * g : 32 * g + 1, :],
                        ones1[:],
                        p4[:, g, :],
                        start=(G == 0),
                        stop=(G == JG - 1),
                        tile_position=(0, 32 * g),
                    )

            # denom partial rows PSUM -> SBUF on ACT (it idles around block
            # boundaries; keeps the DVE queue off the PE critical path)
            def emit_den_finalize(den_ps):
                for g in range(4):
                    nc.scalar.copy(
                        den4_sb[32 * g : 32 * g + 1, :],
                        den_ps[32 * g : 32 * g + 1, :],
                    )

            # ---- Prologue: q/k/vo projections interleaved with D1(ib=0) ----
            # D1(0, G) needs k columns of its own j-tiles (= k-proj ic G) and
            # q columns 0:512 (= q-proj ic 0), so the stagger below keeps PE,
            # ACT and DVE all busy from the start.
            den_ps_cur = ps_den.tile([P, IB], F32, tag="den", name="den_ps")
            p4s_cur = []
            for G in range(JG):
                emit_qk_proj(k_sb, wkt_sb, bk_sb, xv_bf, G)
                emit_qk_proj(q_sb, wqt_sb, bq_sb, xy_bf, G)
                for g in range(4):
                    emit_vo_proj(4 * G + g)
                emit_d1_group(0, G, den_ps_cur, p4s_cur)
            emit_den_finalize(den_ps_cur)

            # ---- Main loop: D2(ib) with D1(ib+1) interleaved ----
            for ib in range(NIB):
                isl = slice(ib * IB, (ib + 1) * IB)
                p4s = p4s_cur

                if ib + 1 < NIB:
                    den_ps_cur = ps_den.tile([P, IB], F32, tag="den", name="den_ps")
                    p4s_cur = []
                    d1_next = [(ib + 1, G) for G in range(JG)]
                else:
                    d1_next = []

                r_sb = pr.tile([P, IB], F32, tag="r", name="r_sb")
                step = 0
                for cc in range(CC):
                    u_ps = ps_u.tile([P, IB], F32, tag="u", name="u_ps")
                    for G in range(JG):
                        for g in range(4):
                            jt = 4 * G + g
                            nc.tensor.matmul(
                                u_ps[:],
                                vo_sb[:, jt, cc * P : (cc + 1) * P],
                                p4s[G][:, g, :],
                                start=(G == 0 and g == 0),
                                stop=(G == JG - 1 and g == 3),
                            )
                        if step == 3:
                            # denom merge: one masked matmul sums rows
                            # {0,32,64,96} of den4_sb and replicates across
                            # all partitions; then reciprocal. Emitted inside
                            # the AV stream so PE never waits on it.
                            rep_ps = ps_misc.tile([P, IB], F32, tag="misc", name="rep_ps")
                            nc.tensor.matmul(
                                rep_ps[:], sel4_sb[:], den4_sb[:], start=True, stop=True
                            )
                            nc.vector.reciprocal(r_sb[:], rep_ps[:])
                        if step % 2 == 1 and d1_next:
                            nib, nG = d1_next.pop(0)
                            emit_d1_group(nib, nG, den_ps_cur, p4s_cur)
                            if not d1_next:
                                emit_den_finalize(den_ps_cur)
                        step += 1
                    ot = pot.tile([P, IB], F32, tag="ot", name="ot")
                    nc.vector.tensor_mul(ot[:], u_ps[:], r_sb[:])
                    nc.vector.tensor_add(ot[:], ot[:], yb[cc][:, isl])
                    nc.sync.dma_start(out[cc * P : (cc + 1) * P, isl], ot[:])

    nc.compile()
    return nc


_NC_CACHE = {}


def _get_nc():
    if "nc" not in _NC_CACHE:
        _NC_CACHE["nc"] = build_nc()
    return _NC_CACHE["nc"]


def _prep_in_maps(inputs):
    yolo = np.ascontiguousarray(np.asarray(inputs["yolo_features"], np.float32))
    vit = np.ascontiguousarray(np.asarray(inputs["vit_features"], np.float32))
    Wq = np.asarray(inputs["Wq"], np.float32)
    bq = np.asarray(inputs["bq"], np.float32)
    Wk = np.asarray(inputs["Wk"], np.float32)
    bk = np.asarray(inputs["bk"], np.float32)
    Wv = np.asarray(inputs["Wv"], np.float32)
    bv = np.asarray(inputs["bv"], np.float32)
    Wo = np.asarray(inputs["Wo"], np.float32)
    bo = np.asarray(inputs["bo"], np.float32)

    wqt = np.ascontiguousarray(Wq.T).astype(BF16)
    wkt = np.ascontiguousarray(Wk.T).astype(BF16)
    wvo = np.ascontiguousarray((Wo @ Wv).T).astype(BF16)
    bq4 = np.ascontiguousarray(np.tile(bq, 4)[:, None].astype(np.float32))
    bk4 = np.ascontiguousarray(np.tile(bk, 4)[:, None].astype(np.float32))
    bop = np.ascontiguousarray((Wo @ bv + bo)[:, None].astype(np.float32))
    sel4 = np.zeros((P, P), dtype=BF16)
    sel4[[0, 32, 64, 96], :] = 1.0

    in_maps = []
    for b in range(B):
        in_maps.append(
            {
                "x_yolo": yolo[b].reshape(C, N),
                "x_vit": vit[b].reshape(C, N),
                "wqt": wqt,
                "wkt": wkt,
                "wvo": wvo,
                "bq4": bq4,
                "bk4": bk4,
                "bop": bop,
                "sel4": sel4,
            }
        )
    return in_maps


def run(inputs, trace=False):
    nc = _get_nc()
    in_maps = _prep_in_maps(inputs)
    res = run_bass_kernel_spmd(nc, in_maps, list(range(B)), trace=trace)
    out = np.stack([res.results[b]["out"] for b in range(B)], axis=0)
    return out.reshape(B, C, H, W).astype(np.float32), res


def kernel(**inputs):
    out, _ = run(inputs, trace=False)
    return out
